# revision 12
# baseline (speedup 1.0000x reference)
"""Multi-head attention Trainium2 kernel (B=8, S=2048, EMB=768, H=4, Dh=192).

Strategy: data-parallel over batch — one batch element per NeuronCore, no
collectives. v2: everything SBUF-resident (no DRAM bounce of Q^T/K^T),
attention software-pipelined so the ACT exp stream always has work, output
projection interleaved per q-block.

Layouts (feature-on-partition everywhere except V):
  P1: QT[do,s] = Wq^T.T @ q^T  -> qt_sb (6 resident [128,2048] tiles); same KT.
      V[s,do] natural          -> v_sb  [128, 16, 4*(192+1)] (+ones col per head)
  P2: per block (qb, h), reading head segments straight out of qt/kt tiles
      via base-partition slicing (Dh=192 = 128 + 64 with tile_position):
        scoresT[k,q] = Kh^T.T @ Qh^T   (k on partitions)
        E = exp(scoresT*scale)          (ACT)
        outT[dh,q] = Vh.T @ E ; Z[q] = ones.T @ E  (rides V's ones column)
        outT *= 1/Z (broadcast via K=1 PE matmul; normalize on DVE into oc)
  P3: out[s,e] = Oc^T.T @ Wo^T (+bo via ones row), per q-block right after
      its 4 heads finish.
"""

import sys

sys.path.insert(0, "/opt/trn_rl_repo")

import numpy as np

import concourse.bass as bass  # noqa: F401  (import keeps bass registered)
import concourse.mybir as mybir
import concourse.tile as tile
from concourse import bacc

B, S, EMB, HEADS = 8, 2048, 768, 4
DH = EMB // HEADS  # 192
NCORES = 8
P = 128
DI_TILES = EMB // P  # 6
S_TILES = S // P  # 16
QBLK = 512
N_QBLK = S // QBLK  # 4
EBLK = 384
SCALE = 1.0 / float(np.sqrt(DH))
VW = DH + 1  # 193 cols per head in the V tile (192 dh + ones)

F32 = mybir.dt.float32
MMDT = mybir.dt.float16  # matmul operand dtype (psum accumulation is fp32)


def _np_mmdt():
    return np.float16


# per-head (seg_tile_idx, row0, row1) pairs covering rows h*DH..(h+1)*DH of
# the 6x[128, S] transposed projection tiles
def _head_segs(h):
    segs = []
    r0 = h * DH
    r1 = (h + 1) * DH
    while r0 < r1:
        j, p0 = divmod(r0, P)
        p1 = min(P, p0 + (r1 - r0))
        segs.append((j, p0, p1))
        r0 += p1 - p0
    return segs


def _build_nc(reps=1, phases=3):
    nc = bacc.Bacc("TRN2", target_bir_lowering=False, debug=False,
                   num_devices=NCORES)

    xq = nc.declare_dram_parameter("xq", [EMB, S], MMDT, isOutput=False)
    xk = nc.declare_dram_parameter("xk", [EMB, S], MMDT, isOutput=False)
    xv = nc.declare_dram_parameter("xv", [EMB + 1, S], MMDT, isOutput=False)
    wq = nc.declare_dram_parameter("wq", [EMB, EMB], MMDT, isOutput=False)
    wk = nc.declare_dram_parameter("wk", [EMB, EMB], MMDT, isOutput=False)
    wv = nc.declare_dram_parameter("wv", [EMB + 1, EMB], MMDT, isOutput=False)
    wo = nc.declare_dram_parameter("wo", [EMB + 1, EMB], MMDT, isOutput=False)
    bq = nc.declare_dram_parameter("bq", [EMB, 1], F32, isOutput=False)
    bk = nc.declare_dram_parameter("bk", [EMB, 1], F32, isOutput=False)
    onesd = nc.declare_dram_parameter("onesd", [P, S], MMDT, isOutput=False)
    out = nc.declare_dram_parameter("out", [S, EMB], F32, isOutput=True)

    with tile.TileContext(nc) as tc:
        with tc.tile_pool(name="res", bufs=1) as res, \
             tc.tile_pool(name="psgen", bufs=4, space="PSUM") as psgen:
            # ---- persistent SBUF residents ----
            kt_sb = [res.tile([P, S], MMDT, name=f"kt{j}", tag=f"kt{j}")
                     for j in range(DI_TILES)]
            qt_sb = [res.tile([P, S], MMDT, name=f"qt{j}", tag=f"qt{j}")
                     for j in range(DI_TILES)]
            v_sb = res.tile([P, S_TILES, HEADS * VW], MMDT, name="v_sb")
            oc_sb = [[res.tile([P, QBLK], MMDT, name=f"oc{j}_{qb}",
                               tag=f"oc{j}_{qb}")
                      for qb in range(N_QBLK)] for j in range(DI_TILES)]
            wo_t = [res.tile([P, EMB], MMDT, name=f"wo{i}", tag=f"wo{i}")
                    for i in range(DI_TILES)]
            wo_last = res.tile([1, EMB], MMDT, name="wol")
            ones_row = res.tile([1, S], MMDT, name="ones_row")
            ones_bcast = res.tile([1, P], MMDT, name="ones_bcast")

            nc.sync.dma_start(out=ones_row, in_=onesd[0:1, :])
            nc.sync.dma_start(out=ones_bcast, in_=onesd[0:1, 0:P])
            # all 4 heads' ones columns in V, one DMA
            nc.sync.dma_start(
                out=v_sb.rearrange("p t (h c) -> p t h c", c=VW)[:, :, :, DH],
                in_=onesd[:, 0:S_TILES * HEADS].rearrange(
                    "p (t h) -> p t h", h=HEADS))
            for i in range(DI_TILES):
                nc.sync.dma_start(out=wo_t[i], in_=wo[i * P:(i + 1) * P, :])
            nc.sync.dma_start(out=wo_last, in_=wo[EMB:EMB + 1, :])

            for rep in range(reps):
                # ============ Phase 1: projections (K, V, Q) ============
                with tc.tile_pool(name=f"w1_{rep}", bufs=1) as wp, \
                     tc.tile_pool(name=f"x1_{rep}", bufs=2) as xp:
                    wk_t = [wp.tile([P, EMB], MMDT, name=f"wk{i}", tag=f"wk{i}")
                            for i in range(DI_TILES)]
                    wq_t = [wp.tile([P, EMB], MMDT, name=f"wq{i}", tag=f"wq{i}")
                            for i in range(DI_TILES)]
                    wv_t = [wp.tile([P, EMB], MMDT, name=f"wv{i}", tag=f"wv{i}")
                            for i in range(DI_TILES)]
                    wv_last = wp.tile([1, EMB], MMDT, name="wvl")
                    bq_t = [wp.tile([P, 1], F32, name=f"bq{i}", tag=f"bq{i}")
                            for i in range(DI_TILES)]
                    bk_t = [wp.tile([P, 1], F32, name=f"bk{i}", tag=f"bk{i}")
                            for i in range(DI_TILES)]
                    for i in range(DI_TILES):
                        nc.sync.dma_start(out=wk_t[i], in_=wk[i * P:(i + 1) * P, :])
                        nc.sync.dma_start(out=bk_t[i], in_=bk[i * P:(i + 1) * P, :])
                    for i in range(DI_TILES):
                        nc.sync.dma_start(out=wv_t[i], in_=wv[i * P:(i + 1) * P, :])
                    nc.sync.dma_start(out=wv_last, in_=wv[EMB:EMB + 1, :])
                    for i in range(DI_TILES):
                        nc.sync.dma_start(out=wq_t[i], in_=wq[i * P:(i + 1) * P, :])
                        nc.sync.dma_start(out=bq_t[i], in_=bq[i * P:(i + 1) * P, :])

                    # K then Q: transposed projections into resident tiles
                    for (xin, wt, bt, dst) in ((xk, wk_t, bk_t, kt_sb),
                                               (xq, wq_t, bq_t, qt_sb)):
                        for sb in range(N_QBLK):
                            scols = slice(sb * QBLK, (sb + 1) * QBLK)
                            xs = []
                            for j in range(DI_TILES):
                                t = xp.tile([P, QBLK], MMDT, name=f"xs{j}",
                                            tag=f"xs{j}")
                                nc.sync.dma_start(out=t, in_=xin[j * P:(j + 1) * P,
                                                                scols])
                                xs.append(t)
                            for do in range(DI_TILES):
                                ps = psgen.tile([P, QBLK], F32, name="gen",
                                                tag="gen")
                                for di in range(DI_TILES):
                                    nc.tensor.matmul(
                                        ps, wt[di][:, do * P:(do + 1) * P],
                                        xs[di][:, :],
                                        start=(di == 0), stop=(di == DI_TILES - 1))
                                with nc.allow_low_precision(
                                        reason="fp16 storage of projections"):
                                    nc.vector.tensor_scalar_add(
                                        dst[do][:, scols], ps, bt[do])

                        if xin is xk:
                            # V projection (natural layout, per-head ones col)
                            for sb in range(N_QBLK):
                                scols = slice(sb * QBLK, (sb + 1) * QBLK)
                                vs = []
                                for j in range(DI_TILES):
                                    t = xp.tile([P, QBLK], MMDT, name=f"vs{j}",
                                                tag=f"vs{j}")
                                    nc.sync.dma_start(
                                        out=t, in_=xv[j * P:(j + 1) * P, scols])
                                    vs.append(t)
                                vlast = xp.tile([1, QBLK], MMDT, name="vsl",
                                                tag="vsl")
                                nc.sync.dma_start(
                                    out=vlast, in_=xv[EMB:EMB + 1, scols])
                                for sti in range(QBLK // P):
                                    st = sb * (QBLK // P) + sti
                                    pcols = slice(sti * P, (sti + 1) * P)
                                    for blk in range(2):  # heads {0,1}, {2,3}
                                        ps = psgen.tile([P, QBLK], F32,
                                                        name="gen", tag="gen")
                                        for di in range(DI_TILES):
                                            nc.tensor.matmul(
                                                ps[:, 0:EBLK], vs[di][:, pcols],
                                                wv_t[di][:, blk * EBLK:(blk + 1) * EBLK],
                                                start=(di == 0), stop=False)
                                        nc.tensor.matmul(
                                            ps[:, 0:EBLK], vlast[:, pcols],
                                            wv_last[:, blk * EBLK:(blk + 1) * EBLK],
                                            start=False, stop=True)
                                        dst = v_sb[:, st,
                                                   blk * 2 * VW:(blk * 2 + 2) * VW]
                                        dst = dst.rearrange(
                                            "p (h c) -> p h c", c=VW)[:, :, 0:DH]
                                        with nc.allow_low_precision(
                                                reason="fp16 storage of V"):
                                            nc.vector.tensor_copy(
                                                dst,
                                                ps[:, 0:EBLK].rearrange(
                                                    "p (h c) -> p h c", c=DH))

                if phases < 2:
                    continue

                # ============ Phase 2+3: pipelined attention ============
                with tc.tile_pool(name=f"ee_{rep}", bufs=2) as eep, \
                     tc.tile_pool(name=f"zz_{rep}", bufs=2) as zzp, \
                     tc.tile_pool(name=f"fe_{rep}", bufs=2) as fep, \
                     tc.tile_pool(name=f"pso1_{rep}", bufs=2, space="PSUM") as pso1, \
                     tc.tile_pool(name=f"pso2_{rep}", bufs=2, space="PSUM") as pso2:

                    blocks = [(qb, h) for qb in range(N_QBLK)
                              for h in range(HEADS)]
                    nb = len(blocks)
                    # live state per in-flight block
                    st_e = {}   # block idx -> e_all tile
                    st_o = {}   # block idx -> (ps_o1, ps_o2)
                    st_rz = {}  # block idx -> rz tile

                    def emit_scores(i):
                        qb, h = blocks[i]
                        qcols = slice(qb * QBLK, (qb + 1) * QBLK)
                        segs = _head_segs(h)
                        e_all = eep.tile([P, S_TILES, QBLK], MMDT,
                                         name="E", tag="E")
                        st_e[i] = e_all
                        for kt in range(S_TILES):
                            kc = slice(kt * P, (kt + 1) * P)
                            ps_e = psgen.tile([P, QBLK], F32, name="gen",
                                              tag="gen")
                            for si, (j, p0, p1) in enumerate(segs):
                                nc.tensor.matmul(
                                    ps_e, kt_sb[j][p0:p1, kc],
                                    qt_sb[j][p0:p1, qcols],
                                    start=(si == 0), stop=(si == len(segs) - 1))
                            nc.scalar.activation(
                                e_all[:, kt, :], ps_e,
                                mybir.ActivationFunctionType.Exp,
                                bias=0.0, scale=SCALE)

                    def emit_attv(i):
                        qb, h = blocks[i]
                        e_all = st_e[i]
                        ps_o1 = pso1.tile([P, QBLK], F32, name="o1", tag="o1")
                        ps_o2 = pso2.tile([DH + 1 - P, QBLK], F32, name="o2",
                                          tag="o2")
                        st_o[i] = (ps_o1, ps_o2)
                        for kt in range(S_TILES):
                            nc.tensor.matmul(
                                ps_o1, v_sb[:, kt, h * VW:h * VW + P],
                                e_all[:, kt, :],
                                start=(kt == 0), stop=(kt == S_TILES - 1))
                            nc.tensor.matmul(
                                ps_o2, v_sb[:, kt, h * VW + P:(h + 1) * VW],
                                e_all[:, kt, :],
                                start=(kt == 0), stop=(kt == S_TILES - 1))
                        rz = zzp.tile([1, QBLK], MMDT, name="rz", tag="rz")
                        st_rz[i] = rz
                        with nc.allow_low_precision(
                                reason="softmax reciprocal, fp16 storage"):
                            nc.vector.reciprocal(rz, ps_o2[DH - P:DH - P + 1, :])

                    def emit_norm(i):
                        qb, h = blocks[i]
                        ps_o1, ps_o2 = st_o.pop(i)
                        rz = st_rz.pop(i)
                        del st_e[i]
                        ps_b = psgen.tile([P, QBLK], F32, name="gen", tag="gen")
                        nc.tensor.matmul(ps_b, ones_bcast[:, :], rz[:, :],
                                         start=True, stop=True)
                        bz = zzp.tile([P, QBLK], F32, name="bz", tag="bz")
                        nc.vector.tensor_copy(bz, ps_b)
                        segs = sorted({0, DH, P} |
                                      {j * P - h * DH for j in range(DI_TILES + 1)
                                       if 0 < j * P - h * DH < DH})
                        with nc.allow_low_precision(
                                reason="softmax normalize, fp16 storage"):
                            for a, b in zip(segs[:-1], segs[1:]):
                                r = h * DH + a
                                j, p0 = divmod(r, P)
                                src = (ps_o1[a:b, :] if b <= P
                                       else ps_o2[a - P:b - P, :])
                                nc.vector.tensor_mul(
                                    oc_sb[j][qb][p0:p0 + (b - a), :],
                                    src, bz[0:b - a, :])

                    def emit_phase3(qb):
                        if phases < 3:
                            return
                        for sti in range(QBLK // P):
                            st = qb * (QBLK // P) + sti
                            scols = slice(st * P, (st + 1) * P)
                            pcols = slice(sti * P, (sti + 1) * P)
                            for eb in range(2):
                                ecols = slice(eb * EBLK, (eb + 1) * EBLK)
                                ps = psgen.tile([P, QBLK], F32, name="gen",
                                                tag="gen")
                                for j in range(DI_TILES):
                                    nc.tensor.matmul(
                                        ps[:, 0:EBLK], oc_sb[j][qb][:, pcols],
                                        wo_t[j][:, ecols],
                                        start=(j == 0), stop=False)
                                nc.tensor.matmul(
                                    ps[:, 0:EBLK], ones_row[0:1, scols],
                                    wo_last[:, ecols],
                                    start=False, stop=True)
                                fin = fep.tile([P, EBLK], F32, name="fin",
                                               tag="fin")
                                nc.vector.tensor_copy(fin, ps[:, 0:EBLK])
                                nc.gpsimd.dma_start(out=out[scols, ecols],
                                                    in_=fin)

                    # software pipeline: scores(i+1) ahead of attV(i);
                    # norm(i) one step behind; phase3 one step behind its
                    # norms so PE never waits on the DVE normalize chain
                    pending_p3 = []
                    emit_scores(0)
                    for i in range(nb):
                        if i + 1 < nb:
                            emit_scores(i + 1)
                        for qb in pending_p3:
                            emit_phase3(qb)
                        pending_p3 = []
                        if i - 1 >= 0:
                            emit_norm(i - 1)
                            qb_prev, h_prev = blocks[i - 1]
                            if h_prev == HEADS - 1:
                                pending_p3.append(qb_prev)
                        emit_attv(i)
                    for qb in pending_p3:
                        emit_phase3(qb)
                    emit_norm(nb - 1)
                    emit_phase3(N_QBLK - 1)

    nc.compile()
    return nc


_CACHE = {}


def _get_runner(reps=1, phases=3):
    """Build nc once and a reusable jitted SPMD callable (no recompiles)."""
    key = f"runner{reps}_{phases}"
    if key in _CACHE:
        return _CACHE[key]

    import jax
    import numpy as _np
    from jax.sharding import Mesh, PartitionSpec
    from jax.experimental.shard_map import shard_map
    from concourse import bass2jax
    from concourse.bass2jax import _bass_exec_p, install_neuronx_cc_hook

    nc = _build_nc(reps, phases)
    install_neuronx_cc_hook()

    partition_name = (nc.partition_id_tensor.name
                      if nc.partition_id_tensor else None)
    in_names, out_names, out_avals, zero_outs = [], [], [], []
    for alloc in nc.m.functions[0].allocations:
        if not isinstance(alloc, mybir.MemoryLocationSet):
            continue
        name = alloc.memorylocations[0].name
        if alloc.kind == "ExternalInput":
            if name != partition_name:
                in_names.append(name)
        elif alloc.kind == "ExternalOutput":
            shape = list(alloc.tensor_shape)
            npdt = mybir.dt.np(alloc.dtype)
            out_avals.append(jax.core.ShapedArray(shape, npdt))
            out_names.append(name)
            zero_outs.append(_np.zeros(shape, npdt))
    n_params = len(in_names)
    n_outs = len(out_names)
    in_names = in_names + out_names
    if partition_name is not None:
        in_names.append(partition_name)

    def _body(*args):
        operands = list(args)
        if partition_name is not None:
            operands.append(bass2jax.partition_id_tensor())
        outs = _bass_exec_p.bind(
            *operands,
            out_avals=tuple(out_avals),
            in_names=tuple(in_names),
            out_names=tuple(out_names),
            lowering_input_output_aliases=(),
            sim_require_finite=True,
            sim_require_nnan=True,
            nc=nc,
        )
        return tuple(outs)

    devices = jax.devices()[:NCORES]
    mesh = Mesh(_np.asarray(devices), ("core",))
    in_specs = (PartitionSpec("core"),) * (n_params + n_outs)
    out_specs = (PartitionSpec("core"),) * n_outs
    sharded = jax.jit(
        shard_map(_body, mesh=mesh, in_specs=in_specs, out_specs=out_specs,
                  check_rep=False),
        keep_unused=True,
    )
    concat_zeros = [
        _np.zeros((NCORES * z.shape[0], *z.shape[1:]), z.dtype)
        for z in zero_outs
    ]

    runner = {
        "nc": nc, "sharded": sharded, "in_names": in_names,
        "n_params": n_params, "out_names": out_names,
        "out_avals": out_avals, "concat_zeros": concat_zeros,
        "mesh": mesh,
    }
    _CACHE[key] = runner
    return runner


def run_spmd(in_maps):
    """Run the compiled SPMD program; in_maps is a list of NCORES dicts."""
    import numpy as _np
    r = _get_runner()
    per_core = [[_np.asarray(m[name]) for name in r["in_names"][:r["n_params"]]]
                for m in in_maps]
    concat_in = [
        _np.concatenate([per_core[c][i] for c in range(NCORES)], axis=0)
        for i in range(r["n_params"])
    ]
    out_arrs = r["sharded"](*concat_in, *r["concat_zeros"])
    return [
        {name: _np.asarray(out_arrs[i]).reshape(NCORES, *r["out_avals"][i].shape)[c]
         for i, name in enumerate(r["out_names"])}
        for c in range(NCORES)
    ]


def _prep_in_maps(q, k, v, Wq, bq, Wk, bk, Wv, bv, Wo, bo):
    mdt = _np_mmdt()
    q = np.asarray(q, dtype=np.float32)
    k = np.asarray(k, dtype=np.float32)
    v = np.asarray(v, dtype=np.float32)
    wqT = np.ascontiguousarray(np.asarray(Wq, np.float32).T).astype(mdt)
    wkT = np.ascontiguousarray(np.asarray(Wk, np.float32).T).astype(mdt)
    wvT = np.ascontiguousarray(
        np.concatenate([np.asarray(Wv, np.float32).T,
                        np.asarray(bv, np.float32)[None, :]],
                       axis=0)).astype(mdt)
    woT = np.ascontiguousarray(
        np.concatenate([np.asarray(Wo, np.float32).T,
                        np.asarray(bo, np.float32)[None, :]],
                       axis=0)).astype(mdt)
    bqc = np.ascontiguousarray(np.asarray(bq, np.float32).reshape(EMB, 1))
    bkc = np.ascontiguousarray(np.asarray(bk, np.float32).reshape(EMB, 1))
    ones = np.ones((P, S), dtype=mdt)
    in_maps = []
    for b in range(NCORES):
        xvT = np.concatenate(
            [v[b].T, np.ones((1, S), np.float32)], axis=0)
        in_maps.append({
            "xq": np.ascontiguousarray(q[b].T).astype(mdt),
            "xk": np.ascontiguousarray(k[b].T).astype(mdt),
            "xv": np.ascontiguousarray(xvT).astype(mdt),
            "wq": wqT, "wk": wkT, "wv": wvT, "wo": woT,
            "bq": bqc, "bk": bkc, "onesd": ones,
        })
    return in_maps


def kernel(q, k, v, Wq, bq, Wk, bk, Wv, bv, Wo, bo):
    in_maps = _prep_in_maps(q, k, v, Wq, bq, Wk, bk, Wv, bv, Wo, bo)
    results = run_spmd(in_maps)
    out = np.stack([results[b]["out"] for b in range(NCORES)], axis=0)
    return out.astype(np.float32)


# revision 19
# speedup vs baseline: 5.0210x; 5.0210x over previous
"""Multi-head attention Trainium2 kernel (B=8, S=2048, EMB=768, H=4, Dh=192).

Strategy: data-parallel over batch — one batch element per NeuronCore, no
collectives. v2: everything SBUF-resident (no DRAM bounce of Q^T/K^T),
attention software-pipelined so the ACT exp stream always has work, output
projection interleaved per q-block.

Layouts (feature-on-partition everywhere except V):
  P1: QT[do,s] = Wq^T.T @ q^T  -> qt_sb (6 resident [128,2048] tiles); same KT.
      V[s,do] natural          -> v_sb  [128, 16, 4*(192+1)] (+ones col per head)
  P2: per block (qb, h), reading head segments straight out of qt/kt tiles
      via base-partition slicing (Dh=192 = 128 + 64 with tile_position):
        scoresT[k,q] = Kh^T.T @ Qh^T   (k on partitions)
        E = exp(scoresT*scale)          (ACT)
        outT[dh,q] = Vh.T @ E ; Z[q] = ones.T @ E  (rides V's ones column)
        outT *= 1/Z (broadcast via K=1 PE matmul; normalize on DVE into oc)
  P3: out[s,e] = Oc^T.T @ Wo^T (+bo via ones row), per q-block right after
      its 4 heads finish.
"""

import sys

sys.path.insert(0, "/opt/trn_rl_repo")

import numpy as np

import concourse.bass as bass  # noqa: F401  (import keeps bass registered)
import concourse.mybir as mybir
import concourse.tile as tile
from concourse import bacc

B, S, EMB, HEADS = 8, 2048, 768, 4
DH = EMB // HEADS  # 192
NCORES = 8
P = 128
DI_TILES = EMB // P  # 6
S_TILES = S // P  # 16
QBLK = 512
N_QBLK = S // QBLK  # 4
EBLK = 384
SCALE = 1.0 / float(np.sqrt(DH))
VW = DH + 1  # 193 cols per head in the V tile (192 dh + ones)

F32 = mybir.dt.float32
MMDT = mybir.dt.float16  # matmul operand dtype (psum accumulation is fp32)


def _np_mmdt():
    return np.float16


# per-head (seg_tile_idx, row0, row1) pairs covering rows h*DH..(h+1)*DH of
# the 6x[128, S] transposed projection tiles
def _head_segs(h):
    segs = []
    r0 = h * DH
    r1 = (h + 1) * DH
    while r0 < r1:
        j, p0 = divmod(r0, P)
        p1 = min(P, p0 + (r1 - r0))
        segs.append((j, p0, p1))
        r0 += p1 - p0
    return segs


def _build_nc(reps=1, phases=3):
    nc = bacc.Bacc("TRN2", target_bir_lowering=False, debug=False,
                   num_devices=NCORES)

    xq = nc.declare_dram_parameter("xq", [EMB, S], MMDT, isOutput=False)
    xk = nc.declare_dram_parameter("xk", [EMB, S], MMDT, isOutput=False)
    xv = nc.declare_dram_parameter("xv", [EMB, S], MMDT, isOutput=False)
    wq = nc.declare_dram_parameter("wq", [EMB, EMB], MMDT, isOutput=False)
    wk = nc.declare_dram_parameter("wk", [EMB, EMB], MMDT, isOutput=False)
    wv = nc.declare_dram_parameter("wv", [EMB, EMB], MMDT, isOutput=False)
    wo = nc.declare_dram_parameter("wo", [EMB, EMB], MMDT, isOutput=False)
    bq = nc.declare_dram_parameter("bq", [EMB, 1], F32, isOutput=False)
    bk = nc.declare_dram_parameter("bk", [EMB, 1], F32, isOutput=False)
    onesd = nc.declare_dram_parameter("onesd", [P, S], MMDT, isOutput=False)
    out = nc.declare_dram_parameter("out", [S, EMB], F32, isOutput=True)

    with tile.TileContext(nc) as tc:
        with tc.tile_pool(name="res", bufs=1) as res, \
             tc.tile_pool(name="psgen", bufs=4, space="PSUM") as psgen:
            # ---- persistent SBUF residents ----
            kt_sb = [res.tile([P, S], MMDT, name=f"kt{j}", tag=f"kt{j}")
                     for j in range(DI_TILES)]
            qt_sb = [res.tile([P, S], MMDT, name=f"qt{j}", tag=f"qt{j}")
                     for j in range(DI_TILES)]
            v_sb = res.tile([P, S_TILES, HEADS * VW], MMDT, name="v_sb")
            wo_t = [res.tile([P, EMB], MMDT, name=f"wo{i}", tag=f"wo{i}")
                    for i in range(DI_TILES)]
            ones_bcast = res.tile([1, P], MMDT, name="ones_bcast")

            nc.sync.dma_start(out=ones_bcast, in_=onesd[0:1, 0:P])
            # all 4 heads' ones columns in V, one DMA
            nc.sync.dma_start(
                out=v_sb.rearrange("p t (h c) -> p t h c", c=VW)[:, :, :, DH],
                in_=onesd[:, 0:S_TILES * HEADS].rearrange(
                    "p (t h) -> p t h", h=HEADS))
            for i in range(DI_TILES):
                nc.sync.dma_start(out=wo_t[i], in_=wo[i * P:(i + 1) * P, :])

            for rep in range(reps):
                # ============ Phase 1: projections (K, V, Q) ============
                with tc.tile_pool(name=f"w1_{rep}", bufs=1) as wp, \
                     tc.tile_pool(name=f"x1_{rep}", bufs=2) as xp:
                    wk_t = [wp.tile([P, EMB], MMDT, name=f"wk{i}", tag=f"wk{i}")
                            for i in range(DI_TILES)]
                    wq_t = [wp.tile([P, EMB], MMDT, name=f"wq{i}", tag=f"wq{i}")
                            for i in range(DI_TILES)]
                    wv_t = [wp.tile([P, EMB], MMDT, name=f"wv{i}", tag=f"wv{i}")
                            for i in range(DI_TILES)]
                    bq_t = [wp.tile([P, 1], F32, name=f"bq{i}", tag=f"bq{i}")
                            for i in range(DI_TILES)]
                    bk_t = [wp.tile([P, 1], F32, name=f"bk{i}", tag=f"bk{i}")
                            for i in range(DI_TILES)]
                    for i in range(DI_TILES):
                        nc.sync.dma_start(out=wk_t[i], in_=wk[i * P:(i + 1) * P, :])
                        nc.sync.dma_start(out=bk_t[i], in_=bk[i * P:(i + 1) * P, :])
                    for i in range(DI_TILES):
                        nc.sync.dma_start(out=wv_t[i], in_=wv[i * P:(i + 1) * P, :])
                    for i in range(DI_TILES):
                        nc.sync.dma_start(out=wq_t[i], in_=wq[i * P:(i + 1) * P, :])
                        nc.sync.dma_start(out=bq_t[i], in_=bq[i * P:(i + 1) * P, :])

                    # K then Q: transposed projections into resident tiles
                    for (xin, wt, bt, dst) in ((xk, wk_t, bk_t, kt_sb),
                                               (xq, wq_t, bq_t, qt_sb)):
                        for sb in range(N_QBLK):
                            scols = slice(sb * QBLK, (sb + 1) * QBLK)
                            xs = []
                            for j in range(DI_TILES):
                                t = xp.tile([P, QBLK], MMDT, name=f"xs{j}",
                                            tag=f"xs{j}")
                                nc.sync.dma_start(out=t, in_=xin[j * P:(j + 1) * P,
                                                                scols])
                                xs.append(t)
                            for do in range(DI_TILES):
                                ps = psgen.tile([P, QBLK], F32, name="gen",
                                                tag="gen")
                                for di in range(DI_TILES):
                                    nc.tensor.matmul(
                                        ps, wt[di][:, do * P:(do + 1) * P],
                                        xs[di][:, :],
                                        start=(di == 0), stop=(di == DI_TILES - 1))
                                with nc.allow_low_precision(
                                        reason="fp16 storage of projections"):
                                    nc.vector.tensor_scalar_add(
                                        dst[do][:, scols], ps, bt[do])

                        if xin is xk:
                            # V projection (natural layout, per-head ones col)
                            for sb in range(N_QBLK):
                                scols = slice(sb * QBLK, (sb + 1) * QBLK)
                                vs = []
                                for j in range(DI_TILES):
                                    t = xp.tile([P, QBLK], MMDT, name=f"vs{j}",
                                                tag=f"vs{j}")
                                    nc.sync.dma_start(
                                        out=t, in_=xv[j * P:(j + 1) * P, scols])
                                    vs.append(t)
                                for sti in range(QBLK // P):
                                    st = sb * (QBLK // P) + sti
                                    pcols = slice(sti * P, (sti + 1) * P)
                                    for blk in range(2):  # heads {0,1}, {2,3}
                                        ps = psgen.tile([P, QBLK], F32,
                                                        name="gen", tag="gen")
                                        for di in range(DI_TILES):
                                            nc.tensor.matmul(
                                                ps[:, 0:EBLK], vs[di][:, pcols],
                                                wv_t[di][:, blk * EBLK:(blk + 1) * EBLK],
                                                start=(di == 0),
                                                stop=(di == DI_TILES - 1))
                                        dst = v_sb[:, st,
                                                   blk * 2 * VW:(blk * 2 + 2) * VW]
                                        dst = dst.rearrange(
                                            "p (h c) -> p h c", c=VW)[:, :, 0:DH]
                                        with nc.allow_low_precision(
                                                reason="fp16 storage of V"):
                                            nc.vector.tensor_copy(
                                                dst,
                                                ps[:, 0:EBLK].rearrange(
                                                    "p (h c) -> p h c", c=DH))

                if phases < 2:
                    continue

                # ============ Phase 2+3: pipelined attention ============
                with tc.tile_pool(name=f"ee_{rep}", bufs=2) as eep, \
                     tc.tile_pool(name=f"zz_{rep}", bufs=2) as zzp, \
                     tc.tile_pool(name=f"fe_{rep}", bufs=2) as fep, \
                     tc.tile_pool(name=f"oc_{rep}", bufs=2) as ocp, \
                     tc.tile_pool(name=f"pso1_{rep}", bufs=2, space="PSUM") as pso1, \
                     tc.tile_pool(name=f"pso2_{rep}", bufs=2, space="PSUM") as pso2:

                    # blocks are (qb, head); scores are emitted per even/odd
                    # head PAIR so the two K=64 tail matmuls land adjacent in
                    # the PE queue on disjoint row groups (rows 0-63 vs
                    # 64-127) and run concurrently.
                    blocks = [(qb, h) for qb in range(N_QBLK)
                              for h in range(HEADS)]
                    nb = len(blocks)
                    # live state per in-flight block
                    st_e = {}   # block idx -> e_all tile
                    st_o = {}   # block idx -> (ps_o1, ps_o2)
                    st_rz = {}  # block idx -> rz tile
                    oc_cur = {}  # qb -> [6 oc tiles]

                    def oc_tiles(qb):
                        if qb not in oc_cur:
                            oc_cur[qb] = [ocp.tile([P, QBLK], MMDT,
                                                   name=f"oc{j}", tag=f"oc{j}")
                                          for j in range(DI_TILES)]
                        return oc_cur[qb]

                    def scores_prep(i):
                        # allocate E tiles for pair starting at even block i
                        e0 = eep.tile([P, S_TILES, QBLK], MMDT, name="E0",
                                      tag="E0")
                        e1 = eep.tile([P, S_TILES, QBLK], MMDT, name="E1",
                                      tag="E1")
                        st_e[i] = e0
                        st_e[i + 1] = e1

                    def scores_kt(i, kt):
                        # one kt of scores for the head pair (i even): the two
                        # K=64 tail matmuls are adjacent on disjoint row groups
                        qb, h0 = blocks[i]
                        qcols = slice(qb * QBLK, (qb + 1) * QBLK)
                        sa = _head_segs(h0)       # [(j, 0, 128), (j', 0, 64)]
                        sb_ = _head_segs(h0 + 1)  # [(j', 64, 128), (j'', 0, 128)]
                        kc = slice(kt * P, (kt + 1) * P)
                        ps_e0 = psgen.tile([P, QBLK], F32, name="gen", tag="gen")
                        ps_e1 = psgen.tile([P, QBLK], F32, name="gen", tag="gen")
                        (ja, a0, a1), (jb, b0, b1) = sa
                        nc.tensor.matmul(ps_e0, kt_sb[ja][a0:a1, kc],
                                         qt_sb[ja][a0:a1, qcols],
                                         start=True, stop=False)
                        nc.tensor.matmul(ps_e0, kt_sb[jb][b0:b1, kc],
                                         qt_sb[jb][b0:b1, qcols],
                                         start=False, stop=True)
                        (jc, c0, c1), (jd, d0, d1) = sb_
                        nc.tensor.matmul(ps_e1, kt_sb[jc][c0:c1, kc],
                                         qt_sb[jc][c0:c1, qcols],
                                         start=True, stop=False)
                        nc.tensor.matmul(ps_e1, kt_sb[jd][d0:d1, kc],
                                         qt_sb[jd][d0:d1, qcols],
                                         start=False, stop=True)
                        nc.scalar.activation(st_e[i][:, kt, :], ps_e0,
                                             mybir.ActivationFunctionType.Exp,
                                             bias=0.0, scale=SCALE)
                        nc.scalar.activation(st_e[i + 1][:, kt, :], ps_e1,
                                             mybir.ActivationFunctionType.Exp,
                                             bias=0.0, scale=SCALE)

                    def attv_prep(i):
                        ps_o1 = pso1.tile([P, QBLK], F32, name="o1", tag="o1")
                        ps_o2 = pso2.tile([DH + 1 - P, QBLK], F32, name="o2",
                                          tag="o2")
                        st_o[i] = (ps_o1, ps_o2)

                    def attv_kt(i, kt):
                        qb, h = blocks[i]
                        e_all = st_e[i]
                        ps_o1, ps_o2 = st_o[i]
                        nc.tensor.matmul(
                            ps_o1, v_sb[:, kt, h * VW:h * VW + P],
                            e_all[:, kt, :],
                            start=(kt == 0), stop=(kt == S_TILES - 1))
                        nc.tensor.matmul(
                            ps_o2, v_sb[:, kt, h * VW + P:(h + 1) * VW],
                            e_all[:, kt, :],
                            start=(kt == 0), stop=(kt == S_TILES - 1))

                    def attv_fin(i):
                        _, ps_o2 = st_o[i]
                        rz = zzp.tile([1, QBLK], MMDT, name="rz", tag="rz")
                        st_rz[i] = rz
                        with nc.allow_low_precision(
                                reason="softmax reciprocal, fp16 storage"):
                            nc.vector.reciprocal(rz, ps_o2[DH - P:DH - P + 1, :])

                    def emit_norm(i):
                        qb, h = blocks[i]
                        ps_o1, ps_o2 = st_o.pop(i)
                        rz = st_rz.pop(i)
                        del st_e[i]
                        ps_b = psgen.tile([P, QBLK], F32, name="gen", tag="gen")
                        nc.tensor.matmul(ps_b, ones_bcast[:, :], rz[:, :],
                                         start=True, stop=True)
                        bz = zzp.tile([P, QBLK], F32, name="bz", tag="bz")
                        nc.vector.tensor_copy(bz, ps_b)
                        oc = oc_tiles(qb)
                        segs = sorted({0, DH, P} |
                                      {j * P - h * DH for j in range(DI_TILES + 1)
                                       if 0 < j * P - h * DH < DH})
                        with nc.allow_low_precision(
                                reason="softmax normalize, fp16 storage"):
                            for a, b in zip(segs[:-1], segs[1:]):
                                r = h * DH + a
                                j, p0 = divmod(r, P)
                                src = (ps_o1[a:b, :] if b <= P
                                       else ps_o2[a - P:b - P, :])
                                nc.vector.tensor_mul(
                                    oc[j][p0:p0 + (b - a), :],
                                    src, bz[0:b - a, :])

                    def emit_phase3(qb):
                        oc = oc_cur.pop(qb)
                        if phases < 3:
                            return
                        for sti in range(QBLK // P):
                            st = qb * (QBLK // P) + sti
                            scols = slice(st * P, (st + 1) * P)
                            pcols = slice(sti * P, (sti + 1) * P)
                            for eb in range(2):
                                ecols = slice(eb * EBLK, (eb + 1) * EBLK)
                                ps = psgen.tile([P, QBLK], F32, name="gen",
                                                tag="gen")
                                for j in range(DI_TILES):
                                    nc.tensor.matmul(
                                        ps[:, 0:EBLK], oc[j][:, pcols],
                                        wo_t[j][:, ecols],
                                        start=(j == 0),
                                        stop=(j == DI_TILES - 1))
                                fin = fep.tile([P, EBLK], F32, name="fin",
                                               tag="fin")
                                nc.vector.tensor_copy(fin, ps[:, 0:EBLK])
                                nc.gpsimd.dma_start(out=out[scols, ecols],
                                                    in_=fin)

                    # software pipeline over head-pairs, merged at kt
                    # granularity: each step interleaves scores(pair p+1)
                    # with attV(pair p) so the ACT exp stream drains into
                    # psgen slots at the same rate PE fills them. attV lags
                    # LAG kts so the norms of pair p-1 (emitted at kt==0)
                    # have released the o-banks before attV reuses them.
                    LAG = 6
                    npair = nb // 2
                    pending_p3 = []
                    scores_prep(0)
                    for kt in range(S_TILES):
                        scores_kt(0, kt)
                    for p in range(npair):
                        has_next = p + 1 < npair
                        if has_next:
                            scores_prep(2 * (p + 1))
                        attv_prep(2 * p)
                        attv_prep(2 * p + 1)
                        nkt = S_TILES + LAG if has_next else S_TILES
                        for kt in range(nkt):
                            if has_next and kt < S_TILES:
                                scores_kt(2 * (p + 1), kt)
                            if kt == 0 and p - 1 >= 0:
                                emit_norm(2 * (p - 1))
                                emit_norm(2 * (p - 1) + 1)
                                qb_prev, h_prev = blocks[2 * (p - 1) + 1]
                                if h_prev == HEADS - 1:
                                    pending_p3.append(qb_prev)
                            akt = kt - LAG if has_next else kt
                            if 0 <= akt < S_TILES:
                                attv_kt(2 * p, akt)
                                attv_kt(2 * p + 1, akt)
                        attv_fin(2 * p)
                        attv_fin(2 * p + 1)
                        for qb in pending_p3:
                            emit_phase3(qb)
                        pending_p3 = []
                    emit_norm(nb - 2)
                    emit_norm(nb - 1)
                    emit_phase3(N_QBLK - 1)

    nc.compile()
    return nc


_CACHE = {}


def _get_runner(reps=1, phases=3):
    """Build nc once and a reusable jitted SPMD callable (no recompiles)."""
    key = f"runner{reps}_{phases}"
    if key in _CACHE:
        return _CACHE[key]

    import jax
    import numpy as _np
    from jax.sharding import Mesh, PartitionSpec
    from jax.experimental.shard_map import shard_map
    from concourse import bass2jax
    from concourse.bass2jax import _bass_exec_p, install_neuronx_cc_hook

    nc = _build_nc(reps, phases)
    install_neuronx_cc_hook()

    partition_name = (nc.partition_id_tensor.name
                      if nc.partition_id_tensor else None)
    in_names, out_names, out_avals, zero_outs = [], [], [], []
    for alloc in nc.m.functions[0].allocations:
        if not isinstance(alloc, mybir.MemoryLocationSet):
            continue
        name = alloc.memorylocations[0].name
        if alloc.kind == "ExternalInput":
            if name != partition_name:
                in_names.append(name)
        elif alloc.kind == "ExternalOutput":
            shape = list(alloc.tensor_shape)
            npdt = mybir.dt.np(alloc.dtype)
            out_avals.append(jax.core.ShapedArray(shape, npdt))
            out_names.append(name)
            zero_outs.append(_np.zeros(shape, npdt))
    n_params = len(in_names)
    n_outs = len(out_names)
    in_names = in_names + out_names
    if partition_name is not None:
        in_names.append(partition_name)

    def _body(*args):
        operands = list(args)
        if partition_name is not None:
            operands.append(bass2jax.partition_id_tensor())
        outs = _bass_exec_p.bind(
            *operands,
            out_avals=tuple(out_avals),
            in_names=tuple(in_names),
            out_names=tuple(out_names),
            lowering_input_output_aliases=(),
            sim_require_finite=True,
            sim_require_nnan=True,
            nc=nc,
        )
        return tuple(outs)

    devices = jax.devices()[:NCORES]
    mesh = Mesh(_np.asarray(devices), ("core",))
    in_specs = (PartitionSpec("core"),) * (n_params + n_outs)
    out_specs = (PartitionSpec("core"),) * n_outs
    sharded = jax.jit(
        shard_map(_body, mesh=mesh, in_specs=in_specs, out_specs=out_specs,
                  check_rep=False),
        keep_unused=True,
    )
    concat_zeros = [
        _np.zeros((NCORES * z.shape[0], *z.shape[1:]), z.dtype)
        for z in zero_outs
    ]

    runner = {
        "nc": nc, "sharded": sharded, "in_names": in_names,
        "n_params": n_params, "out_names": out_names,
        "out_avals": out_avals, "concat_zeros": concat_zeros,
        "mesh": mesh,
    }
    _CACHE[key] = runner
    return runner


def run_spmd(in_maps):
    """Run the compiled SPMD program; in_maps is a list of NCORES dicts."""
    import numpy as _np
    r = _get_runner()
    per_core = [[_np.asarray(m[name]) for name in r["in_names"][:r["n_params"]]]
                for m in in_maps]
    concat_in = [
        _np.concatenate([per_core[c][i] for c in range(NCORES)], axis=0)
        for i in range(r["n_params"])
    ]
    out_arrs = r["sharded"](*concat_in, *r["concat_zeros"])
    return [
        {name: _np.asarray(out_arrs[i]).reshape(NCORES, *r["out_avals"][i].shape)[c]
         for i, name in enumerate(r["out_names"])}
        for c in range(NCORES)
    ]


def _prep_in_maps(q, k, v, Wq, bq, Wk, bk, Wv, bv, Wo, bo):
    mdt = _np_mmdt()
    q = np.asarray(q, dtype=np.float32)
    k = np.asarray(k, dtype=np.float32)
    v = np.asarray(v, dtype=np.float32)
    wqT = np.ascontiguousarray(np.asarray(Wq, np.float32).T).astype(mdt)
    wkT = np.ascontiguousarray(np.asarray(Wk, np.float32).T).astype(mdt)
    wvT = np.ascontiguousarray(np.asarray(Wv, np.float32).T).astype(mdt)
    woT = np.ascontiguousarray(np.asarray(Wo, np.float32).T).astype(mdt)
    bqc = np.ascontiguousarray(np.asarray(bq, np.float32).reshape(EMB, 1))
    bkc = np.ascontiguousarray(np.asarray(bk, np.float32).reshape(EMB, 1))
    ones = np.ones((P, S), dtype=mdt)
    in_maps = []
    for b in range(NCORES):
        in_maps.append({
            "xq": np.ascontiguousarray(q[b].T).astype(mdt),
            "xk": np.ascontiguousarray(k[b].T).astype(mdt),
            "xv": np.ascontiguousarray(v[b].T).astype(mdt),
            "wq": wqT, "wk": wkT, "wv": wvT, "wo": woT,
            "bq": bqc, "bk": bkc, "onesd": ones,
        })
    return in_maps


def kernel(q, k, v, Wq, bq, Wk, bk, Wv, bv, Wo, bo):
    in_maps = _prep_in_maps(q, k, v, Wq, bq, Wk, bk, Wv, bv, Wo, bo)
    results = run_spmd(in_maps)
    out = np.stack([results[b]["out"] for b in range(NCORES)], axis=0)
    out = out.astype(np.float32)
    # exact epilogue: softmax rows sum to 1, so the V bias contributes
    # bv @ Wo.T to every output row; fold it with bo on the host.
    extra = (np.asarray(bo, np.float32)
             + np.asarray(Wo, np.float32) @ np.asarray(bv, np.float32))
    if np.any(extra):
        out = out + extra
    return out
